# revision 5
# baseline (speedup 1.0000x reference)
"""Focal Gaussian loss (EDT heatmap + focal MSE) on 8 Trainium2 cores.

Data-parallel over batch: each core processes B/8 = 2 images end to end
(row EDT via DVE scans, column min-conv via fused scalar_tensor_tensor
taps with a +-T window, exp heatmap, focal MSE), producing per-image
partial sums (sum of focal_factor, sum of focal*mse). The host combines
the 8 cores' partials and applies the global normalization:

    out = SCALE * mean(focal*mse) / (mean(focal) + 0.01)

The +-T window on the column pass is exact wherever the resulting
heatmap is above ~exp(-T^2/8); beyond that both reference and kernel
heatmaps are ~0 at fp32, so the final scalar is unaffected.
"""

import numpy as np

B, H, W = 16, 512, 512
N_CORES = 8
IPC = B // N_CORES  # images per core
T = 16              # column min-conv window radius
BIG = 1.0e6
BIG2 = 1.0e12
SCALE = 2.0
EPS = 0.01
P = 128
RB = H // P  # row blocks
CB = W // P  # col blocks
WPAD = 512 + 2 * T

_CACHE = {}


def build_program():
    import concourse.bacc as bacc
    import concourse.mybir as mybir
    import concourse.tile as tile

    f32 = mybir.dt.float32
    Alu = mybir.AluOpType
    Act = mybir.ActivationFunctionType

    nc = bacc.Bacc(
        "TRN2", target_bir_lowering=False, debug=False, num_devices=N_CORES
    )

    inp_d = nc.dram_tensor("inputs", [IPC, H, W], f32, kind="ExternalInput").ap()
    tgt_d = nc.dram_tensor("targets", [IPC, H, W], f32, kind="ExternalInput").ap()
    cplus_d = nc.dram_tensor("cplus", [P, W], f32, kind="ExternalInput").ap()
    cminus_d = nc.dram_tensor("cminus", [P, W], f32, kind="ExternalInput").ap()
    ident_d = nc.dram_tensor("ident", [P, P], f32, kind="ExternalInput").ap()
    part_d = nc.dram_tensor("partials", [P, 2 * IPC], f32, kind="ExternalOutput").ap()

    with tile.TileContext(nc) as tc:
        with (
            tc.tile_pool(name="const", bufs=1) as cpool,
            tc.tile_pool(name="io", bufs=2) as iopool,
            tc.tile_pool(name="work", bufs=2) as wpool,
            tc.tile_pool(name="psum", bufs=4, space="PSUM") as ppool,
            tc.tile_pool(name="outp", bufs=1) as opool,
        ):
            cplus = cpool.tile([P, W], f32)
            nc.sync.dma_start(cplus[:], cplus_d[:])
            cminus = cpool.tile([P, W], f32)
            nc.sync.dma_start(cminus[:], cminus_d[:])
            ident = cpool.tile([P, P], f32)
            nc.sync.dma_start(ident[:], ident_d[:])
            bias015 = cpool.tile([P, 1], f32)
            nc.vector.memset(bias015[:], 0.15)
            partials = opool.tile([P, 2 * IPC], f32)

            for img in range(IPC):
                tgt = iopool.tile([P, RB, W], f32, tag="tgt")
                nc.sync.dma_start(
                    tgt[:], tgt_d[img].rearrange("(a p) w -> p a w", p=P)
                )
                inp = iopool.tile([P, RB, W], f32, tag="inp")
                nc.sync.dma_start(
                    inp[:], inp_d[img].rearrange("(a p) w -> p a w", p=P)
                )

                # --- horizontal pass (row-major): f2 = squared distance to
                # nearest fg pixel within the row, reference cummax/cummin trick
                f2 = wpool.tile([P, RB, W], f32, tag="f2")
                for rb in range(RB):
                    trb = tgt[:, rb]
                    u0 = wpool.tile([P, W], f32, tag="u0")
                    v0 = wpool.tile([P, W], f32, tag="v0")
                    nc.vector.tensor_mul(u0[:], trb, cplus[:])
                    nc.vector.tensor_mul(v0[:], trb, cminus[:])
                    scl = wpool.tile([P, W], f32, tag="scl")
                    scr = wpool.tile([P, W], f32, tag="scr")
                    nc.vector.tensor_tensor_scan(
                        scl[:], u0[:], u0[:], 0.0, Alu.max, Alu.max
                    )
                    nc.vector.tensor_tensor_scan(
                        scr[:, ::-1], v0[:, ::-1], v0[:, ::-1], 0.0, Alu.max, Alu.max
                    )
                    # reuse u0/v0 for the two distances, scl for their min
                    nc.vector.tensor_sub(u0[:], cplus[:], scl[:])
                    nc.vector.tensor_sub(v0[:], cminus[:], scr[:])
                    nc.vector.tensor_tensor(scl[:], u0[:], v0[:], op=Alu.min)
                    nc.scalar.square(f2[:, rb], scl[:])

                # --- transpose f2 to column-major with +-T row padding
                f2T = wpool.tile([P, CB, WPAD], f32, tag="f2T")
                nc.vector.memset(f2T[:, :, 0:T], BIG2)
                nc.vector.memset(f2T[:, :, T + 512 : WPAD], BIG2)
                for cb in range(CB):
                    ps = ppool.tile([P, 512], f32, tag="psT")
                    for rb in range(RB):
                        nc.tensor.transpose(
                            ps[:, rb * P : (rb + 1) * P],
                            f2[:, rb, cb * P : (cb + 1) * P],
                            ident[:],
                        )
                    nc.scalar.copy(f2T[:, cb, T : T + 512], ps[:])

                # --- vertical min-conv: d2[i] = min_t (t^2 + f2[i+t])
                acc = wpool.tile([P, CB, 512], f32, tag="acc")
                nc.vector.scalar_tensor_tensor(
                    acc[:],
                    f2T[:, :, T + 1 : T + 1 + 512],
                    1.0,
                    f2T[:, :, T : T + 512],
                    Alu.add,
                    Alu.min,
                )
                for t in list(range(-T, 0)) + list(range(2, T + 1)):
                    nc.vector.scalar_tensor_tensor(
                        acc[:],
                        f2T[:, :, T + t : T + t + 512],
                        float(t * t),
                        acc[:],
                        Alu.add,
                        Alu.min,
                    )

                # --- transpose back, fused with heat = exp(-d2/8)
                heat = wpool.tile([P, RB, W], f32, tag="heat")
                for rb in range(RB):
                    ph = ppool.tile([P, 512], f32, tag="psH")
                    for cb in range(CB):
                        nc.tensor.transpose(
                            ph[:, cb * P : (cb + 1) * P],
                            acc[:, cb, rb * P : (rb + 1) * P],
                            ident[:],
                        )
                    nc.scalar.activation(
                        heat[:, rb], ph[:], Act.Exp, scale=-0.125
                    )

                # --- focal MSE; tgt is exactly 0/1 so the where()s reduce to
                # arithmetic: 1-pt = pred + pos - 2*pos*pred,
                # alpha_t = 0.7*pos + 0.15
                pred = wpool.tile([P, RB, W], f32, tag="pred")
                nc.scalar.activation(pred[:], inp[:], Act.Sigmoid)
                q_ = wpool.tile([P, RB, W], f32, tag="q_")
                nc.vector.tensor_mul(q_[:], tgt[:], pred[:])
                nc.vector.scalar_tensor_tensor(
                    q_[:], q_[:], -2.0, pred[:], Alu.mult, Alu.add
                )
                nc.vector.tensor_add(q_[:], q_[:], tgt[:])
                nc.scalar.square(q_[:], q_[:])
                alpha = wpool.tile([P, RB, W], f32, tag="alpha")
                nc.scalar.activation(
                    alpha[:], tgt[:], Act.Identity, bias=bias015[:], scale=0.7
                )
                # focal overwrites alpha; q_ holds (1-pt)^2 here
                nc.vector.scalar_tensor_tensor(
                    alpha[:],
                    alpha[:],
                    1.0,
                    q_[:],
                    Alu.mult,
                    Alu.mult,
                    accum_out=partials[:, 2 * img : 2 * img + 1],
                )
                # d = pred - heat overwrites pred, then squared in place
                nc.vector.tensor_sub(pred[:], pred[:], heat[:])
                nc.vector.tensor_mul(pred[:], pred[:], pred[:])
                nc.vector.scalar_tensor_tensor(
                    q_[:],
                    alpha[:],
                    1.0,
                    pred[:],
                    Alu.mult,
                    Alu.mult,
                    accum_out=partials[:, 2 * img + 1 : 2 * img + 2],
                )

            nc.sync.dma_start(part_d[:], partials[:])

    nc.compile()
    return nc


def host_constants():
    cols = np.arange(W, dtype=np.float32)
    cplus = np.broadcast_to(cols + BIG, (P, W)).copy()
    cminus = np.broadcast_to(BIG - cols, (P, W)).copy()
    ident = np.eye(P, dtype=np.float32)
    return cplus, cminus, ident


def make_in_maps(inputs, targets):
    cplus, cminus, ident = host_constants()
    in_maps = []
    for c in range(N_CORES):
        sl = slice(c * IPC, (c + 1) * IPC)
        in_maps.append(
            {
                "inputs": np.ascontiguousarray(inputs[sl, 0]),
                "targets": np.ascontiguousarray(targets[sl, 0]),
                "cplus": cplus,
                "cminus": cminus,
                "ident": ident,
            }
        )
    return in_maps


def combine_partials(partial_list):
    """partial_list: one [128, 2*IPC] array per core -> final scalar."""
    sf = 0.0
    sl = 0.0
    for parts in partial_list:
        p64 = parts.astype(np.float64)
        sf += p64[:, 0::2].sum()
        sl += p64[:, 1::2].sum()
    n = float(B * H * W)
    out = SCALE * (sl / n) / (sf / n + EPS)
    return np.float32(out)


def kernel(inputs, targets):
    from concourse.bass_utils import run_bass_kernel_spmd

    if "nc" not in _CACHE:
        _CACHE["nc"] = build_program()
    nc = _CACHE["nc"]

    in_maps = make_in_maps(inputs, targets)
    res = run_bass_kernel_spmd(nc, in_maps, list(range(N_CORES)))
    return combine_partials([r["partials"] for r in res.results])
